# revision 1
# baseline (speedup 1.0000x reference)
"""HQQ int4 weight-only quantized linear (tinygemm convention) on 8 trn2 cores.

out[b,s,o] = sum_k x[b,s,k] * ((Wq[o,k]-8)*scales[o,g(k)] + zeros[o,g(k)]) + bias[o]

Sharding: column-parallel — W_q/scales/zeros/bias split along out_features (11008)
into 8 shards of 1376 (padded to 1408 = 11 tiles of 128); x replicated.

Per-core algorithm (out^T orientation, o on partitions):
  out^T[o,m] = sum_g s[o,g] * P_g[o,m] + sum_g c[o,g]*t[g,m] + bias[o]
  where P_g = Wq_g^T-block @ x_g^T (raw int4 values as bf16, PE matmul),
        c = zeros - 8*scales, t[g,m] = sum_{k in g} x[m,k].
The per-group scale is applied to PSUM partials on the vector engine with a
stride-0-broadcast multiplier; the c/bias correction is one extra rank-33 matmul.
"""

import sys

sys.path.insert(0, "/opt/trn_rl_repo")

import numpy as np
import ml_dtypes

from concourse import bacc, tile, mybir
from concourse.bass_utils import run_bass_kernel_spmd

BF16 = mybir.dt.bfloat16
F32 = mybir.dt.float32
I32 = mybir.dt.int32

N_CORES = 8
B = 32          # batch (B*S = 32 rows of x)
K = 4096        # in_features
O_TOTAL = 11008
O_C = O_TOTAL // N_CORES   # 1376 out_features per core
O_PAD = 1408               # padded to 11 * 128
T_OT = O_PAD // 128        # 11 o-tiles
NG = 32                    # number of K-groups (group size 128)
GS = 128                   # group size


def build_program():
    nc = bacc.Bacc("TRN2", target_bir_lowering=False, debug=False,
                   num_devices=N_CORES)

    wqt_d = nc.dram_tensor("wqt", [NG, 128, O_PAD], I32, kind="ExternalInput")
    xt_d = nc.dram_tensor("xt", [128, NG, B], BF16, kind="ExternalInput")
    scn_d = nc.dram_tensor("scn", [128, T_OT, NG], BF16, kind="ExternalInput")
    st_d = nc.dram_tensor("st", [NG, O_PAD], BF16, kind="ExternalInput")
    zt_d = nc.dram_tensor("zt", [NG, O_PAD], BF16, kind="ExternalInput")
    bias_d = nc.dram_tensor("biasr", [1, O_PAD], BF16, kind="ExternalInput")
    bmat_d = nc.dram_tensor("bmat", [128, NG, NG], BF16, kind="ExternalInput")
    out_d = nc.dram_tensor("out", [128, T_OT, B], BF16, kind="ExternalOutput")

    with tile.TileContext(nc) as tc:
        with (
            tc.tile_pool(name="const", bufs=1) as const,
            tc.tile_pool(name="wpool", bufs=4) as wpool,
            tc.tile_pool(name="tmppool", bufs=3) as tmppool,
            tc.tile_pool(name="pg", bufs=4, space="PSUM") as pg,
            tc.tile_pool(name="pt", bufs=1, space="PSUM") as pt,
            tc.tile_pool(name="pt2", bufs=1, space="PSUM") as pt2,
        ):
            xt_sb = const.tile([128, NG, B], BF16)
            nc.sync.dma_start(xt_sb[:], xt_d.ap())
            bmat_sb = const.tile([128, NG, NG], BF16)
            nc.sync.dma_start(bmat_sb[:], bmat_d.ap())
            scn_sb = const.tile([128, T_OT, NG], BF16)
            nc.sync.dma_start(scn_sb[:], scn_d.ap())
            st_sb = const.tile([NG, O_PAD], BF16)
            nc.sync.dma_start(st_sb[:], st_d.ap())
            zt_sb = const.tile([NG, O_PAD], BF16)
            nc.sync.dma_start(zt_sb[:], zt_d.ap())

            # c rows = zeros - 8*scales; row 32 = bias (for the ones-row of t_aug)
            ct_sb = const.tile([NG + 1, O_PAD], BF16)
            nc.vector.scalar_tensor_tensor(
                ct_sb[0:NG, :], st_sb[:], -8.0, zt_sb[:],
                op0=mybir.AluOpType.mult, op1=mybir.AluOpType.add,
            )
            nc.sync.dma_start(ct_sb[NG:NG + 1, :], bias_d.ap())

            acc = const.tile([128, T_OT, B], F32)
            nc.vector.memset(acc[:], 0.0)

            # t[g, m] = sum_{k in group g} x^T[k, m]  (accumulated over k-tiles)
            pt_t = pt.tile([NG, B], F32)
            for kt in range(NG):
                nc.tensor.matmul(
                    pt_t[:], bmat_sb[:, kt, :], xt_sb[:, kt, :],
                    start=(kt == 0), stop=(kt == NG - 1),
                )
            t_aug = const.tile([NG + 1, B], BF16)
            nc.vector.tensor_copy(t_aug[0:NG, :], pt_t[:])
            nc.vector.memset(t_aug[NG:NG + 1, :], 1.0)

            # term2[o, m] = sum_g c[o,g] t[g,m] + bias[o]
            pt2_t = pt2.tile([128, T_OT, B], F32)
            for ot in range(T_OT):
                nc.tensor.matmul(
                    pt2_t[:, ot, :], ct_sb[:, ot * 128:(ot + 1) * 128], t_aug[:],
                    start=True, stop=True,
                )

            # main loop over K-groups
            for g in range(NG):
                w_sb = wpool.tile([128, O_PAD], BF16)
                nc.gpsimd.dma_start(w_sb[:], wqt_d.ap()[g])  # int32 -> bf16 cast
                pg_t = pg.tile([128, T_OT, B], F32)
                for ot in range(T_OT):
                    nc.tensor.matmul(
                        pg_t[:, ot, :], w_sb[:, ot * 128:(ot + 1) * 128],
                        xt_sb[:, g, :], start=True, stop=True,
                    )
                tmp_t = tmppool.tile([128, T_OT, B], F32)
                scb = scn_sb[:, :, g].unsqueeze(2).broadcast_to([128, T_OT, B])
                nc.vector.tensor_tensor(tmp_t[:], pg_t[:], scb,
                                        mybir.AluOpType.mult)
                nc.vector.tensor_tensor(acc[:], acc[:], tmp_t[:],
                                        mybir.AluOpType.add)

            out_sb = const.tile([128, T_OT, B], BF16)
            nc.vector.tensor_tensor(out_sb[:], acc[:], pt2_t[:],
                                    mybir.AluOpType.add)
            nc.sync.dma_start(out_d.ap(), out_sb[:])

    nc.compile()
    return nc


_NC_CACHE = None


def _get_program():
    global _NC_CACHE
    if _NC_CACHE is None:
        _NC_CACHE = build_program()
    return _NC_CACHE


def make_in_maps(x, W_q, scales, zeros, bias):
    """Shard + restage inputs for the 8 cores. Pure layout permutation."""
    bf = ml_dtypes.bfloat16
    x2 = np.asarray(x).reshape(B, K)
    # xt[p, g, m] = x2[m, 128*g + p]
    xt = np.ascontiguousarray(
        np.asarray(x2, dtype=bf).T.reshape(NG, GS, B).transpose(1, 0, 2))
    bmat = np.ascontiguousarray(
        np.broadcast_to(np.eye(NG, dtype=bf), (128, NG, NG)))

    in_maps = []
    for c in range(N_CORES):
        sl = slice(c * O_C, (c + 1) * O_C)
        wq_s = np.zeros((O_PAD, K), dtype=np.int32)
        wq_s[:O_C] = np.asarray(W_q)[sl]
        sc_s = np.zeros((O_PAD, NG), dtype=bf)
        sc_s[:O_C] = np.asarray(scales)[sl]
        zr_s = np.zeros((O_PAD, NG), dtype=bf)
        zr_s[:O_C] = np.asarray(zeros)[sl]
        bi_s = np.zeros((1, O_PAD), dtype=bf)
        bi_s[0, :O_C] = np.asarray(bias)[sl]

        # wqt[g, p, o] = W_q[o, 128*g + p]
        wqt = np.ascontiguousarray(wq_s.T).reshape(NG, GS, O_PAD)
        # scn[p, ot, g] = scales[128*ot + p, g]
        scn = np.ascontiguousarray(
            sc_s.reshape(T_OT, 128, NG).transpose(1, 0, 2))
        st = np.ascontiguousarray(sc_s.T)
        zt = np.ascontiguousarray(zr_s.T)

        in_maps.append({
            "wqt": wqt, "xt": xt, "scn": scn, "st": st, "zt": zt,
            "biasr": bi_s, "bmat": bmat,
        })
    return in_maps


def unshard(results):
    parts = []
    for c in range(N_CORES):
        arr = np.asarray(results[c]["out"])          # [128, 11, 32]
        outT = arr.transpose(1, 0, 2).reshape(O_PAD, B)[:O_C]
        parts.append(outT)
    full = np.concatenate(parts, axis=0)             # [11008, 32]
    return np.ascontiguousarray(full.T).reshape(B, 1, O_TOTAL)


def kernel(x, W_q, scales, zeros, bias):
    nc = _get_program()
    in_maps = make_in_maps(x, W_q, scales, zeros, bias)
    res = run_bass_kernel_spmd(nc, in_maps, list(range(N_CORES)))
    return unshard(res.results)


# revision 4
# speedup vs baseline: 26.0633x; 26.0633x over previous
"""HQQ int4 weight-only quantized linear (tinygemm convention) on 8 trn2 cores.

out[b,s,o] = sum_k x[b,s,k] * ((Wq[o,k]-8)*scales[o,g(k)] + zeros[o,g(k)]) + bias[o]

Sharding: column-parallel — W_q/scales/zeros/bias split along out_features (11008)
into 8 shards of 1376 (padded to 1408 = 11 tiles of 128); x replicated.

Per-core algorithm (out^T orientation, o on partitions):
  out^T[o,m] = sum_g s[o,g] * P_g[o,m] + sum_g c[o,g]*t[g,m] + bias[o]
  where P_g = Wq_g^T-block @ x_g^T (raw int4 values as bf16, PE matmul),
        c = zeros - 8*scales, t[g,m] = sum_{k in g} x[m,k].
The per-group scale is applied to PSUM partials on the vector engine with a
stride-0-broadcast multiplier; the c/bias correction is one extra rank-33 matmul.
"""

import sys

sys.path.insert(0, "/opt/trn_rl_repo")

import numpy as np
import ml_dtypes

from concourse import bacc, tile, mybir
from concourse.bass_utils import run_bass_kernel_spmd

BF16 = mybir.dt.bfloat16
F32 = mybir.dt.float32
I32 = mybir.dt.int32

N_CORES = 8
B = 32          # batch (B*S = 32 rows of x)
K = 4096        # in_features
O_TOTAL = 11008
O_C = O_TOTAL // N_CORES   # 1376 out_features per core
O_PAD = 1408               # padded to 11 * 128
T_OT = O_PAD // 128        # 11 o-tiles
NG = 32                    # number of K-groups (group size 128)
GS = 128                   # group size


def build_program(repeat=1):
    nc = bacc.Bacc("TRN2", target_bir_lowering=False, debug=False,
                   num_devices=N_CORES)

    wqt_d = nc.dram_tensor("wqt", [NG, 128, O_PAD], I32, kind="ExternalInput")
    xt_d = nc.dram_tensor("xt", [128, NG, B], BF16, kind="ExternalInput")
    scn_d = nc.dram_tensor("scn", [128, T_OT, NG], BF16, kind="ExternalInput")
    st_d = nc.dram_tensor("st", [NG, O_PAD], BF16, kind="ExternalInput")
    zt_d = nc.dram_tensor("zt", [NG, O_PAD], BF16, kind="ExternalInput")
    bias_d = nc.dram_tensor("biasr", [1, O_PAD], BF16, kind="ExternalInput")
    bmat_d = nc.dram_tensor("bmat", [128, NG, NG], BF16, kind="ExternalInput")
    out_d = nc.dram_tensor("out", [128, T_OT, B], BF16, kind="ExternalOutput")

    with tile.TileContext(nc) as tc:
      for _rep in range(repeat):
        with (
            tc.tile_pool(name="const", bufs=1) as const,
            tc.tile_pool(name="wpool", bufs=4) as wpool,
            tc.tile_pool(name="tmppool", bufs=3) as tmppool,
            tc.tile_pool(name="pg", bufs=4, space="PSUM") as pg,
            tc.tile_pool(name="pt", bufs=1, space="PSUM") as pt,
            tc.tile_pool(name="pt2", bufs=1, space="PSUM") as pt2,
        ):
            xt_sb = const.tile([128, NG, B], BF16)
            nc.sync.dma_start(xt_sb[:], xt_d.ap())
            bmat_sb = const.tile([128, NG, NG], BF16)
            nc.sync.dma_start(bmat_sb[:], bmat_d.ap())
            scn_sb = const.tile([128, T_OT, NG], BF16)
            nc.sync.dma_start(scn_sb[:], scn_d.ap())
            st_sb = const.tile([NG, O_PAD], BF16)
            nc.sync.dma_start(st_sb[:], st_d.ap())
            zt_sb = const.tile([NG, O_PAD], BF16)
            nc.sync.dma_start(zt_sb[:], zt_d.ap())

            # c rows = zeros - 8*scales; row 32 = bias (for the ones-row of t_aug)
            ct_sb = const.tile([NG + 1, O_PAD], BF16)
            nc.vector.scalar_tensor_tensor(
                ct_sb[0:NG, :], st_sb[:], -8.0, zt_sb[:],
                op0=mybir.AluOpType.mult, op1=mybir.AluOpType.add,
            )
            nc.sync.dma_start(ct_sb[NG:NG + 1, :], bias_d.ap())

            acc = const.tile([128, T_OT, B], F32)
            nc.vector.memset(acc[:], 0.0)

            # t[g, m] = sum_{k in group g} x^T[k, m]  (accumulated over k-tiles)
            pt_t = pt.tile([NG, B], F32)
            for kt in range(NG):
                nc.tensor.matmul(
                    pt_t[:], bmat_sb[:, kt, :], xt_sb[:, kt, :],
                    start=(kt == 0), stop=(kt == NG - 1),
                )
            t_aug = const.tile([NG + 1, B], BF16)
            nc.vector.tensor_copy(t_aug[0:NG, :], pt_t[:])
            nc.vector.memset(t_aug[NG:NG + 1, :], 1.0)

            # term2[o, m] = sum_g c[o,g] t[g,m] + bias[o]
            pt2_t = pt2.tile([128, T_OT, B], F32)
            for ot in range(T_OT):
                nc.tensor.matmul(
                    pt2_t[:, ot, :], ct_sb[:, ot * 128:(ot + 1) * 128], t_aug[:],
                    start=True, stop=True,
                )

            # main loop over K-groups
            for g in range(NG):
                w_sb = wpool.tile([128, O_PAD], BF16)
                nc.gpsimd.dma_start(w_sb[:], wqt_d.ap()[g])  # int32 -> bf16 cast
                pg_t = pg.tile([128, T_OT, B], F32)
                for ot in range(T_OT):
                    nc.tensor.matmul(
                        pg_t[:, ot, :], w_sb[:, ot * 128:(ot + 1) * 128],
                        xt_sb[:, g, :], start=True, stop=True,
                    )
                tmp_t = tmppool.tile([128, T_OT, B], F32)
                scb = scn_sb[:, :, g].unsqueeze(2).broadcast_to([128, T_OT, B])
                nc.vector.tensor_tensor(tmp_t[:], pg_t[:], scb,
                                        mybir.AluOpType.mult)
                nc.vector.tensor_tensor(acc[:], acc[:], tmp_t[:],
                                        mybir.AluOpType.add)

            out_sb = const.tile([128, T_OT, B], BF16)
            nc.vector.tensor_tensor(out_sb[:], acc[:], pt2_t[:],
                                    mybir.AluOpType.add)
            nc.sync.dma_start(out_d.ap(), out_sb[:])

    nc.compile()
    return nc


_NC_CACHE = {}


def _get_program(repeat=1):
    if repeat not in _NC_CACHE:
        _NC_CACHE[repeat] = build_program(repeat)
    return _NC_CACHE[repeat]


def make_in_maps(x, W_q, scales, zeros, bias):
    """Shard + restage inputs for the 8 cores. Pure layout permutation."""
    bf = ml_dtypes.bfloat16
    x2 = np.asarray(x).reshape(B, K)
    # xt[p, g, m] = x2[m, 128*g + p]
    xt = np.ascontiguousarray(
        np.asarray(x2, dtype=bf).T.reshape(NG, GS, B).transpose(1, 0, 2))
    bmat = np.ascontiguousarray(
        np.broadcast_to(np.eye(NG, dtype=bf), (128, NG, NG)))

    in_maps = []
    for c in range(N_CORES):
        sl = slice(c * O_C, (c + 1) * O_C)
        wq_s = np.zeros((O_PAD, K), dtype=np.int32)
        wq_s[:O_C] = np.asarray(W_q)[sl]
        sc_s = np.zeros((O_PAD, NG), dtype=bf)
        sc_s[:O_C] = np.asarray(scales)[sl]
        zr_s = np.zeros((O_PAD, NG), dtype=bf)
        zr_s[:O_C] = np.asarray(zeros)[sl]
        bi_s = np.zeros((1, O_PAD), dtype=bf)
        bi_s[0, :O_C] = np.asarray(bias)[sl]

        # wqt[g, p, o] = W_q[o, 128*g + p]
        wqt = np.ascontiguousarray(wq_s.T).reshape(NG, GS, O_PAD)
        # scn[p, ot, g] = scales[128*ot + p, g]
        scn = np.ascontiguousarray(
            sc_s.reshape(T_OT, 128, NG).transpose(1, 0, 2))
        st = np.ascontiguousarray(sc_s.T)
        zt = np.ascontiguousarray(zr_s.T)

        in_maps.append({
            "wqt": wqt, "xt": xt, "scn": scn, "st": st, "zt": zt,
            "biasr": bi_s, "bmat": bmat,
        })
    return in_maps


def unshard(results):
    parts = []
    for c in range(N_CORES):
        arr = np.asarray(results[c]["out"])          # [128, 11, 32]
        outT = arr.transpose(1, 0, 2).reshape(O_PAD, B)[:O_C]
        parts.append(outT)
    full = np.concatenate(parts, axis=0)             # [11008, 32]
    return np.ascontiguousarray(full.T).reshape(B, 1, O_TOTAL)


def kernel(x, W_q, scales, zeros, bias):
    nc = _get_program()
    in_maps = make_in_maps(x, W_q, scales, zeros, bias)
    res = run_bass_kernel_spmd(nc, in_maps, list(range(N_CORES)))
    return unshard(res.results)
